# revision 3
# baseline (speedup 1.0000x reference)
"""AMICO ADMM solver on 8 TRN2 NeuronCores.

min_x ||y - A x||^2 + lambda*|x|_1, x >= 0 via ADMM (100 iterations),
data-parallel over voxels (1024 voxels per core).

Single-state reformulation (rho=1, kappa=lambda/rho):
  With v := x + u (post x-update), the reference iteration collapses to
    s' = |v - kappa|          (s' = (z - u) + kappa, the matmul input)
    m' = min(v, kappa)        (m = u)
    v' = W @ s' + D + m'      (D = W@AtY - kappa*(W@1) constant, host f64)
  PSUM layout per iteration: one [128, 2048] tile (4 banks) holding v for
  all four (r, c) regions. Per region the PE runs
    W[kc0] @ s'_0, W[kc1] @ s'_1, I @ mu      (mu := D + m, fp16, injected)
  so psum == v directly; no PSUM-source vector add is ever needed.
  ScalarE:  s' = Abs(psum - kappa)            (PSUM -> SBUF fp16, 570 ns/chunk)
  VectorE:  mu' = (psum min kappa) add D      (fused scalar_tensor_tensor,
                                               one [128,1024] op per r-block)
  Iterations 0 and 99 inject D itself (m_0 = 0; the last psum must be x_100,
  not v_100), so the final output is a plain PSUM->SBUF copy.

Engine budget per iteration (errata-adjusted): PE 12 matmuls x 518 cyc
@2.4GHz = 2.59 us (bottleneck); ScalarE 4 x 570 ns = 2.28 us; VectorE
2 x 1.19 us = 2.38 us. The old design was VectorE-bound at 3.4 us.
"""

import os

import numpy as np

M = 256
K = 256
N_VOX = 8192
N_CORES = 8
N_SHARD = N_VOX // N_CORES  # 1024
RHO = 1.0
LAMBDA_REG = 0.1
KAPPA = LAMBDA_REG / RHO
N_ITERS = 100

LAST_RESULTS = None  # BassKernelResults of the most recent run (for test.py)


def _build_graph():
    import concourse.mybir as mybir
    from concourse import bacc
    from concourse.tile import TileContext

    f32 = mybir.dt.float32
    fp16 = mybir.dt.float16
    kap = float(KAPPA)

    nc = bacc.Bacc("TRN2", target_bir_lowering=False, debug=False)

    # D16[p, r*1024 + n] = D[r*128+p, n]   (host f64 -> fp16)
    D16_p = nc.declare_dram_parameter("D16", [128, 2048], fp16, isOutput=False)
    # W16[p, kc*256 + r*128 + j] = W[kc*128+p, r*128+j]  (W symmetric)
    W16_p = nc.declare_dram_parameter("W16", [128, 512], fp16, isOutput=False)
    I_p = nc.declare_dram_parameter("ident", [128, 128], fp16, isOutput=False)
    # out[p, r*1024 + n] = x[r*128+p, n]
    O_p = nc.declare_dram_parameter("out", [128, 2048], f32, isOutput=True)

    abs_f = mybir.ActivationFunctionType.Abs
    copy_f = mybir.ActivationFunctionType.Copy
    alu_min = mybir.AluOpType.min
    alu_add = mybir.AluOpType.add

    def reg(r, c):
        return slice(r * 1024 + c * 512, r * 1024 + c * 512 + 512)

    def rblk(r):
        return slice(r * 1024, r * 1024 + 1024)

    with TileContext(nc) as tc:
        with (
            tc.tile_pool(name="static", bufs=1) as statics,
            tc.tile_pool(name="spool", bufs=8) as spool,
            tc.tile_pool(name="mpool", bufs=4) as mpool,
            tc.tile_pool(name="psum_loop", bufs=2, space="PSUM") as psl,
        ):
            D16_sb = statics.tile([128, 2048], fp16, name="D16_sb")
            nc.sync.dma_start(D16_sb[:, :], D16_p[:, :])
            W16_sb = statics.tile([128, 512], fp16, name="W16_sb")
            nc.sync.dma_start(W16_sb[:, :], W16_p[:, :])
            i_sb = statics.tile([128, 128], fp16, name="i_sb")
            nc.sync.dma_start(i_sb[:, :], I_p[:, :])
            out_sb = statics.tile([128, 2048], f32, name="out_sb")
            nkapb_sb = statics.tile([128, 1], f32, name="nkapb_sb")
            nc.vector.memset(nkapb_sb[:, :], -kap)
            # D again in fp32: STT's in0 (PSUM, fp32) and in1 share a dtype.
            D32_sb = statics.tile([128, 2048], f32, name="D32_sb")
            nc.vector.tensor_copy(D32_sb[:, :], D16_sb[:, :])

            # Tiny dummy activations up front so the ~2.7us ACT_TABLE_LOAD
            # overlaps the input DMAs instead of stalling iteration 1.
            warm_sb = statics.tile([1, 8], f32, name="warm_sb")
            nc.scalar.activation(
                warm_sb[:, :], nkapb_sb[:1, :].to_broadcast((1, 8)), abs_f,
                bias=nkapb_sb[:1, :], scale=1.0,
            )
            nc.scalar.activation(
                warm_sb[:, :], warm_sb[:, :], copy_f, bias=0.0, scale=1.0,
            )

            # ---- init: s'_0 = kappa (fp16); mu_0 = D (inject D16 directly) ----
            s_h = [[None, None], [None, None]]
            for kc in (0, 1):
                for c in (0, 1):
                    s0 = spool.tile([128, 512], fp16, name="s_new", tag="s")
                    nc.vector.memset(s0[:, :], kap)
                    s_h[kc][c] = s0
            m_h = [None, None]  # [128, 1024] fp16 per r-block, from STT

            # ---- 100 ADMM iterations, fully unrolled ----
            for it in range(N_ITERS):
                last = it == N_ITERS - 1
                ps = psl.tile([128, 2048], f32, name="ps_v", tag="ps")
                for r in (0, 1):  # r-major: r-block finishes early for the STT
                    for c in (0, 1):
                        dst = ps[:, reg(r, c)]
                        for kc in (0, 1):
                            w0 = kc * 256 + r * 128
                            nc.tensor.matmul(
                                dst,
                                W16_sb[:, w0 : w0 + 128],
                                s_h[kc][c][:, :],
                                start=(kc == 0),
                                stop=False,
                                skip_group_check=True,
                            )
                        if it == 0 or last:
                            inj = D16_sb[:, reg(r, c)]
                        else:
                            inj = m_h[r][:, c * 512 : c * 512 + 512]
                        nc.tensor.matmul(
                            dst,
                            i_sb[:, :],
                            inj,
                            start=False,
                            stop=True,
                            skip_group_check=True,
                        )

                if last:
                    # psum == x_100; copy out on both ScalarE and VectorE.
                    for c in (0, 1):
                        nc.scalar.activation(
                            out_sb[:, reg(0, c)], ps[:, reg(0, c)], copy_f,
                            bias=0.0, scale=1.0,
                        )
                        nc.vector.tensor_copy(out_sb[:, reg(1, c)], ps[:, reg(1, c)])
                    for r in (0, 1):
                        for c in (0, 1):
                            nc.sync.dma_start(O_p[:, reg(r, c)], out_sb[:, reg(r, c)])
                    break

                new_s = [[None, None], [None, None]]
                new_m = [None, None]
                for r in (0, 1):
                    for c in (0, 1):
                        sn = spool.tile([128, 512], fp16, name="s_new", tag="s")
                        nc.scalar.activation(
                            sn[:, :], ps[:, reg(r, c)], abs_f,
                            bias=nkapb_sb[:, :], scale=1.0,
                        )
                        new_s[r][c] = sn
                    if it < N_ITERS - 2:
                        mn = mpool.tile([128, 1024], fp16, name="m_new", tag="m")
                        nc.vector.scalar_tensor_tensor(
                            mn[:, :], ps[:, rblk(r)], kap, D32_sb[:, rblk(r)],
                            alu_min, alu_add,
                        )
                        new_m[r] = mn
                s_h, m_h = new_s, new_m

    nc.compile()
    return nc


_GRAPH = None


def kernel(A: np.ndarray, data: np.ndarray) -> np.ndarray:
    global _GRAPH, LAST_RESULTS
    from concourse.bass_utils import run_bass_kernel_spmd

    A = np.ascontiguousarray(np.asarray(A, dtype=np.float32))
    data = np.ascontiguousarray(np.asarray(data, dtype=np.float32))
    assert A.shape == (M, K) and data.shape == (N_VOX, M)

    # Host-side precompute in f64:
    #   W = (AtA + rho I)^-1 (symmetric), D = W@AtY - kappa*(W@1).
    A64 = A.astype(np.float64)
    AtA = A64.T @ A64
    W = np.linalg.inv(AtA + RHO * np.eye(K))
    w1 = KAPPA * (W @ np.ones(K))

    W_dev = (
        W.astype(np.float32).reshape(2, 128, K).transpose(1, 0, 2).reshape(128, 2 * K)
    )
    W16_dev = W_dev.astype(np.float16)
    i_dev = np.eye(128, dtype=np.float16)

    in_maps = []
    for i in range(N_CORES):
        shard = data[i * N_SHARD : (i + 1) * N_SHARD]  # [1024, 256]
        AtY = A64.T @ shard.astype(np.float64).T  # [256, 1024]
        D = (W @ AtY) - w1[:, None]  # [256, 1024] f64
        D_dev = (
            D.astype(np.float16)
            .reshape(2, 128, N_SHARD)
            .transpose(1, 0, 2)
            .reshape(128, 2 * N_SHARD)
        )
        in_maps.append(
            {
                "D16": np.ascontiguousarray(D_dev),
                "W16": W16_dev,
                "ident": i_dev,
            }
        )

    if _GRAPH is None:
        _GRAPH = _build_graph()

    trace = bool(int(os.environ.get("KERNEL_TRACE", "0")))
    res = run_bass_kernel_spmd(
        _GRAPH, in_maps, core_ids=list(range(N_CORES)), trace=trace
    )
    LAST_RESULTS = res

    out = np.empty((N_VOX, K), dtype=np.float32)
    for i in range(N_CORES):
        o = res.results[i]["out"]  # [128, 2048]
        X = o.reshape(128, 2, N_SHARD).transpose(1, 0, 2).reshape(K, N_SHARD)
        out[i * N_SHARD : (i + 1) * N_SHARD] = X.T
    return out


# revision 7
# speedup vs baseline: 1.1246x; 1.1246x over previous
"""AMICO ADMM solver on 8 TRN2 NeuronCores.

min_x ||y - A x||^2 + lambda*|x|_1, x >= 0 via ADMM (100 iterations),
data-parallel over voxels (1024 voxels per core).

Reformulation (rho=1, kappa=lambda/rho): with v := x + u and t := v - kappa,
the reference iteration is t' = W@|t| + D + min(t,0), D = W@AtY - kappa*W@1.
Using min(t,0) = t/2 - |t|/2 and folding -|t|/2 into the WEIGHTS:

    t' = (W - I/2) @ s'  +  (I/2) @ (t + 2D)        s' := |t|

Per [128,512] psum region the PE runs [Wt[kc0]@s'; Wt[kc1]@s'; (I/2)@tD]
(12 matmuls/iter, 213 ns each = the 2.56 us/iter floor).  The entire
elementwise budget per iteration is then:
  readout t (PSUM -> fp16 SBUF, each element ONCE):
      ScalarE Copy-act for regions q0,q2; VectorE tensor_copy for q1,q3
  s' = t & 0x7fff   (uint16-bitcast tensor_scalar, 4x mode, 2 wide ops, V)
  tD = t + 2D       (fp16 tensor_tensor add, 2 wide ops, GpSimd -- its only
                     job; SBUF-only so Pool can run it; fallback: VectorE)
There is NO min/mu computation left.  The last iteration uses the original
W weights and injects (I/2)@(2D), so psum == x_100 and the output is a
plain PSUM->SBUF copy.  Iteration 0 injects tD_0 = 2D - kappa (s'_0 = kappa).

Engine budget (trace-calibrated): PE 2.56 us (bottleneck), ScalarE 1.64,
VectorE 2.20, GpSimd 2 wide TT-adds.  Schedule validated by discrete-event
simulation: steady period == PE floor when the GpSimd op <= ~1 us.
"""

import os

import numpy as np

M = 256
K = 256
N_VOX = 8192
N_CORES = 8
N_SHARD = N_VOX // N_CORES  # 1024
RHO = 1.0
LAMBDA_REG = 0.1
KAPPA = LAMBDA_REG / RHO
N_ITERS = 100

USE_GPSIMD = bool(int(os.environ.get("KERNEL_GPSIMD", "1")))

LAST_RESULTS = None  # BassKernelResults of the most recent run (for test.py)

# region q -> (r, c): q0=(0,0) q1=(1,0) q2=(0,1) q3=(1,1)
Q_RC = [(0, 0), (1, 0), (0, 1), (1, 1)]


def _build_graph():
    import concourse.mybir as mybir
    from concourse import bacc
    from concourse.tile import TileContext

    f32 = mybir.dt.float32
    fp16 = mybir.dt.float16
    u16 = mybir.dt.uint16
    kap = float(KAPPA)

    nc = bacc.Bacc("TRN2", target_bir_lowering=False, debug=False)

    # D2[p, q*512 + j] = 2*D[r(q)*128+p, c(q)*512+j]   (host f64 -> fp16)
    D2_p = nc.declare_dram_parameter("D2", [128, 2048], fp16, isOutput=False)
    # Wt16[p, kc*256 + r*128 + j] = (W - I/2)[kc*128+p, r*128+j]
    Wt_p = nc.declare_dram_parameter("Wt16", [128, 512], fp16, isOutput=False)
    # Wo16: original W, used only by the final iteration
    Wo_p = nc.declare_dram_parameter("Wo16", [128, 512], fp16, isOutput=False)
    Ih_p = nc.declare_dram_parameter("identh", [128, 128], fp16, isOutput=False)
    # out[p, q*512 + j] = x[r(q)*128+p, c(q)*512+j]
    O_p = nc.declare_dram_parameter("out", [128, 2048], f32, isOutput=True)

    copy_f = mybir.ActivationFunctionType.Copy
    alu_and = mybir.AluOpType.bitwise_and

    def q_sl(q):
        return slice(q * 512, q * 512 + 512)

    td_engine = "gpsimd" if USE_GPSIMD else "vector"

    with TileContext(nc) as tc:
        with (
            tc.tile_pool(name="static", bufs=1) as statics,
            tc.tile_pool(name="spool", bufs=4) as spool,
            tc.tile_pool(name="tpool", bufs=2) as tpool,
            tc.tile_pool(name="tdpool", bufs=4) as tdpool,
            tc.tile_pool(name="psum_loop", bufs=2, space="PSUM") as psl,
        ):
            D2_sb = statics.tile([128, 2048], fp16, name="D2_sb")
            nc.sync.dma_start(D2_sb[:, :], D2_p[:, :])
            Wt_sb = statics.tile([128, 512], fp16, name="Wt_sb")
            nc.sync.dma_start(Wt_sb[:, :], Wt_p[:, :])
            Wo_sb = statics.tile([128, 512], fp16, name="Wo_sb")
            nc.sync.dma_start(Wo_sb[:, :], Wo_p[:, :])
            ih_sb = statics.tile([128, 128], fp16, name="ih_sb")
            nc.sync.dma_start(ih_sb[:, :], Ih_p[:, :])
            out_sb = statics.tile([128, 2048], f32, name="out_sb")
            # per-partition u16 mask 0x7fff for the bitvec abs
            mask_sb = statics.tile([128, 1], u16, name="mask_sb")
            nc.vector.memset(mask_sb[:, :], 0x7FFF)

            # Tiny dummy Copy activation so the ACT_TABLE_LOAD overlaps the
            # input DMAs instead of stalling iteration 1.
            warm_sb = statics.tile([1, 8], f32, name="warm_sb")
            nc.vector.memset(warm_sb[:, :], 0.0)
            nc.scalar.activation(
                warm_sb[:, :], warm_sb[:, :], copy_f, bias=0.0, scale=1.0,
            )

            # tD_0 = 2D - kappa (fp16)
            td0_sb = statics.tile([128, 2048], fp16, name="td0_sb")
            nc.vector.tensor_scalar_sub(td0_sb[:, :], D2_sb[:, :], kap)

            # ---- init: s'_0 = kappa (fp16) ----
            s_c = [None, None]  # per column half: [128,1024] = [kc0|kc1]
            for c in (0, 1):
                s0 = spool.tile([128, 1024], fp16, name="s_new", tag="s")
                nc.vector.memset(s0[:, :], kap)
                s_c[c] = s0
            td_w = [None, None]  # [128,1024] fp16: [q0|q1], [q2|q3]

            # ---- 100 ADMM iterations, fully unrolled ----
            for it in range(N_ITERS):
                last = it == N_ITERS - 1
                W_sb = Wo_sb if last else Wt_sb
                ps = psl.tile([128, 2048], f32, name="ps_t", tag="ps")
                for q, (r, c) in enumerate(Q_RC):
                    dst = ps[:, q_sl(q)]
                    for kc in (0, 1):
                        w0 = kc * 256 + r * 128
                        nc.tensor.matmul(
                            dst,
                            W_sb[:, w0 : w0 + 128],
                            s_c[c][:, kc * 512 : kc * 512 + 512],
                            start=(kc == 0),
                            stop=False,
                            skip_group_check=True,
                        )
                    if last:
                        inj = D2_sb[:, q_sl(q)]
                    elif it == 0:
                        inj = td0_sb[:, q_sl(q)]
                    else:
                        inj = td_w[q // 2][:, (q % 2) * 512 : (q % 2) * 512 + 512]
                    nc.tensor.matmul(
                        dst, ih_sb[:, :], inj,
                        start=False, stop=True, skip_group_check=True,
                    )

                if last:
                    # psum == x_100; copy out on both ScalarE and VectorE.
                    for q in (0, 1):
                        nc.scalar.activation(
                            out_sb[:, q_sl(q)], ps[:, q_sl(q)], copy_f,
                            bias=0.0, scale=1.0,
                        )
                    for q in (2, 3):
                        nc.vector.tensor_copy(out_sb[:, q_sl(q)], ps[:, q_sl(q)])
                    for q in range(4):
                        nc.sync.dma_start(O_p[:, q_sl(q)], out_sb[:, q_sl(q)])
                    break

                # ---- t readout: PSUM -> fp16 SBUF, each element ONCE ----
                t_sb = tpool.tile([128, 2048], fp16, name="t_sb", tag="t")
                nc.scalar.activation(
                    t_sb[:, q_sl(0)], ps[:, q_sl(0)], copy_f, bias=0.0, scale=1.0
                )
                nc.vector.tensor_copy(t_sb[:, q_sl(1)], ps[:, q_sl(1)])
                nc.scalar.activation(
                    t_sb[:, q_sl(2)], ps[:, q_sl(2)], copy_f, bias=0.0, scale=1.0
                )
                # V: abs for column 0 (wide [128,1024] over q0|q1)
                sn0 = spool.tile([128, 1024], fp16, name="s_new", tag="s")
                nc.vector.tensor_scalar(
                    sn0[:, :].bitcast(u16),
                    t_sb[:, 0:1024].bitcast(u16),
                    mask_sb[:, :],
                    None,
                    alu_and,
                )
                # V: readout q3
                nc.vector.tensor_copy(t_sb[:, q_sl(3)], ps[:, q_sl(3)])
                # G (or V): tD for q0|q1 (not needed by the final iteration)
                td0 = None
                if it < N_ITERS - 2:
                    td0 = tdpool.tile([128, 1024], fp16, name="td_new", tag="td")
                    getattr(nc, td_engine).tensor_add(
                        td0[:, :], t_sb[:, 0:1024], D2_sb[:, 0:1024]
                    )
                # V: abs for column 1 (wide over q2|q3)
                sn1 = spool.tile([128, 1024], fp16, name="s_new", tag="s")
                nc.vector.tensor_scalar(
                    sn1[:, :].bitcast(u16),
                    t_sb[:, 1024:2048].bitcast(u16),
                    mask_sb[:, :],
                    None,
                    alu_and,
                )
                # G (or V): tD for q2|q3 (not needed by the final iteration)
                td2 = None
                if it < N_ITERS - 2:
                    td2 = tdpool.tile([128, 1024], fp16, name="td_new", tag="td")
                    getattr(nc, td_engine).tensor_add(
                        td2[:, :], t_sb[:, 1024:2048], D2_sb[:, 1024:2048]
                    )

                s_c = [sn0, sn1]
                td_w = [td0, td2]

    nc.compile()
    return nc


_GRAPH = None


def kernel(A: np.ndarray, data: np.ndarray) -> np.ndarray:
    global _GRAPH, LAST_RESULTS
    from concourse.bass_utils import run_bass_kernel_spmd

    A = np.ascontiguousarray(np.asarray(A, dtype=np.float32))
    data = np.ascontiguousarray(np.asarray(data, dtype=np.float32))
    assert A.shape == (M, K) and data.shape == (N_VOX, M)

    # Host-side precompute in f64:
    #   W = (AtA + rho I)^-1 (symmetric), D = W@AtY - kappa*(W@1).
    A64 = A.astype(np.float64)
    AtA = A64.T @ A64
    W = np.linalg.inv(AtA + RHO * np.eye(K))
    w1 = KAPPA * (W @ np.ones(K))
    Wt = W - 0.5 * np.eye(K)

    def w_layout(Wm):
        return np.ascontiguousarray(
            Wm.astype(np.float32)
            .reshape(2, 128, K)
            .transpose(1, 0, 2)
            .reshape(128, 2 * K)
            .astype(np.float16)
        )

    Wt_dev = w_layout(Wt)
    Wo_dev = w_layout(W)
    ih_dev = np.ascontiguousarray(0.5 * np.eye(128, dtype=np.float16))

    in_maps = []
    for i in range(N_CORES):
        shard = data[i * N_SHARD : (i + 1) * N_SHARD]  # [1024, 256]
        AtY = A64.T @ shard.astype(np.float64).T  # [256, 1024]
        D = (W @ AtY) - w1[:, None]  # [256, 1024] f64
        D2 = (2.0 * D).astype(np.float16)
        # q-major layout: D2_dev[:, q*512+j] = 2D[r(q)*128+p, c(q)*512+j]
        D2_dev = np.empty((128, 2048), dtype=np.float16)
        for q, (r, c) in enumerate(Q_RC):
            D2_dev[:, q * 512 : (q + 1) * 512] = D2[
                r * 128 : (r + 1) * 128, c * 512 : (c + 1) * 512
            ]
        in_maps.append(
            {
                "D2": np.ascontiguousarray(D2_dev),
                "Wt16": Wt_dev,
                "Wo16": Wo_dev,
                "identh": ih_dev,
            }
        )

    if _GRAPH is None:
        _GRAPH = _build_graph()

    trace = bool(int(os.environ.get("KERNEL_TRACE", "0")))
    res = run_bass_kernel_spmd(
        _GRAPH, in_maps, core_ids=list(range(N_CORES)), trace=trace
    )
    LAST_RESULTS = res

    out = np.empty((N_VOX, K), dtype=np.float32)
    for i in range(N_CORES):
        o = res.results[i]["out"]  # [128, 2048] q-major
        for q, (r, c) in enumerate(Q_RC):
            blk = o[:, q * 512 : (q + 1) * 512]  # x[r*128+p, c*512+j]
            out[i * N_SHARD + c * 512 : i * N_SHARD + c * 512 + 512,
                r * 128 : (r + 1) * 128] = blk.T
    return out
